# revision 18
# baseline (speedup 1.0000x reference)
"""DNC-cell scan kernel for Trainium2 (8 NeuronCores, data-parallel over batch).

Reference semantics (per timestep, batch row b):
    att    = concat([mem, x_t[:, None, :]], axis=1)          # [B, M+1, U]
    read_w = softmax(att.mean(1) @ Wr + br)                  # [B, M+1]
    read   = einsum('bm,bmu->bu', read_w, att)               # [B, U]
    tr     = relu(read @ Wt + bt)                            # [B, U]
    ww     = sigmoid(tr @ Ww + bw)[..., None]                # [B, M, 1]
    mem    = (1-ww)*mem + ww*tr[:, None, :]
    y_t    = x_t + tr

Sharding: batch 64 -> 8 cores x 8 rows. All weights replicated; the
time scan stays local per core. Host pre-transposes x/mem0 to U-major
so the kernel needs no on-chip input transposes.

On-chip layout (per core, BS=8 batch rows):
  mem tile  [128(p=u%128), 4(uc), 8(b), 49]  slots: [0:16] mem, [16] x_t,
            [17:33] tmp2 = ww*tr_bc, [33:49] tmp1 = (1-ww)*mem
  The next step's 17-mean reduces slots [16:49] (= x_{t+1} + tmp1 + tmp2),
  which lets mem' = tmp1+tmp2 (u3, on GPSIMD) run off the critical path.

Softmax is computed unnormalized: e = exp(logits); the 1/sum(e) factor is
applied as a per-partition ACT scale on the (batch-major) relu input, since
q @ Wt is linear in q.

Per-partition->free broadcasts use two tricks:
  *  e/ww broadcast across partitions: R = e (x) I8 (one VE op) followed by
     ones[8,128]^T @ R on the PE (sum over the injected identity index
     reproduces e on every partition).
  *  per-b scalars in batch-major layouts are ACT per-partition scales.
"""

import os
import sys
import numpy as np

B, T, U, M = 64, 256, 512, 16
NCORES = 8
BS = B // NCORES          # 8 batch rows per core
UC = U // 128             # 4 partition chunks of the U axis
MP1 = M + 1               # 17

_REPO_CANDIDATES = ("/opt/trn_rl_repo", os.path.expanduser("~/trn_rl_repo"))


def _ensure_import_paths():
    try:
        import concourse  # noqa: F401
        return
    except ImportError:
        pass
    for p in _REPO_CANDIDATES:
        if os.path.isdir(p) and p not in sys.path:
            sys.path.insert(0, p)
    import concourse  # noqa: F401


_NC_CACHE = {}


def _build(tsteps, use_br, use_bt, use_bw):
    """Build the per-core Bass program. Same program runs SPMD on all cores."""
    _ensure_import_paths()
    from contextlib import ExitStack

    import concourse.tile as tile
    from concourse import bacc, mybir

    f32 = mybir.dt.float32
    AF = mybir.ActivationFunctionType
    AX = mybir.AxisListType

    nc = bacc.Bacc()

    xT = nc.declare_dram_parameter("xT", [U, BS, tsteps], f32, isOutput=False)
    memT = nc.declare_dram_parameter("memT", [U, BS, M], f32, isOutput=False)
    WrS = nc.declare_dram_parameter("WrS", [U, MP1], f32, isOutput=False)
    Wt = nc.declare_dram_parameter("Wt", [U, U], f32, isOutput=False)
    Ww = nc.declare_dram_parameter("Ww", [U, M], f32, isOutput=False)
    ones8 = nc.declare_dram_parameter("ones8", [BS, 128], f32, isOutput=False)
    I8 = nc.declare_dram_parameter("I8", [BS, BS], f32, isOutput=False)
    if use_br:
        ebr8 = nc.declare_dram_parameter("ebr8", [BS, MP1], f32, isOutput=False)
    if use_bt:
        bt8 = nc.declare_dram_parameter("bt8", [BS, U], f32, isOutput=False)
    if use_bw:
        bw8 = nc.declare_dram_parameter("bw8", [BS, M], f32, isOutput=False)
    yT = nc.declare_dram_parameter("yT", [U, BS, tsteps], f32, isOutput=True)

    with tile.TileContext(nc) as tc, ExitStack() as ctx:
        const = ctx.enter_context(tc.tile_pool(name="const", bufs=1))
        state = ctx.enter_context(tc.tile_pool(name="state", bufs=1))
        work = ctx.enter_context(tc.tile_pool(name="work", bufs=2))
        ppool = ctx.enter_context(tc.tile_pool(name="ppool", bufs=1, space="PSUM"))

        # ---- constants / weights ----
        Wr_sb = const.tile([128, UC, MP1], f32)
        nc.gpsimd.dma_start(Wr_sb[:], WrS[:, :].rearrange("(k p) j -> p k j", p=128))
        Wt_sb = const.tile([128, UC, U], f32)
        nc.gpsimd.dma_start(Wt_sb[:], Wt[:, :].rearrange("(k p) n -> p k n", p=128))
        Ww_sb = const.tile([128, UC, M], f32)
        nc.gpsimd.dma_start(Ww_sb[:], Ww[:, :].rearrange("(k p) j -> p k j", p=128))
        ones8_sb = const.tile([BS, 128], f32)
        nc.gpsimd.dma_start(ones8_sb[:], ones8[:, :])
        I8_sb = const.tile([BS, BS], f32)
        nc.gpsimd.dma_start(I8_sb[:], I8[:, :])
        if use_br:
            ebr_sb = const.tile([BS, MP1], f32)
            nc.gpsimd.dma_start(ebr_sb[:], ebr8[:, :])
        if use_bt:
            bt_sb = const.tile([BS, U], f32)
            nc.gpsimd.dma_start(bt_sb[:], bt8[:, :])
        if use_bw:
            bw_sb = const.tile([BS, M], f32)
            nc.gpsimd.dma_start(bw_sb[:], bw8[:, :])

        # ---- big state tiles ----
        xs = state.tile([128, UC, BS, tsteps], f32)
        xT_r = xT[:, :, :].rearrange("(k p) b t -> p k b t", p=128)
        for k in range(UC):
            nc.gpsimd.dma_start(xs[:, k, :, :], xT_r[:, k, :, :])
        ys = state.tile([128, UC, BS, tsteps], f32)
        mem = state.tile([128, UC, BS, 49], f32)
        memT_r = memT[:, :, :].rearrange("(k p) b m -> p k b m", p=128)
        for k in range(UC):
            nc.gpsimd.dma_start(mem[:, k, :, 0:M], memT_r[:, k, :, :])

        # Wait-limit workaround: the HW allows only a couple of sync waits
        # per instruction, but a first consumer of several DMA-loaded
        # regions would need one wait per DMA queue. Absorb each DMA's
        # semaphore in its own tiny probe op on every engine that reads
        # the region, so real instructions carry <=2 waits.
        probe = const.tile([128, 16], f32)
        pi = 0
        for k in range(UC):
            nc.vector.tensor_copy(probe[:, pi:pi + 1], xs[:, k, 0, 0:1])
            pi += 1
            nc.vector.tensor_copy(probe[:, pi:pi + 1], mem[:, k, 0, 0:1])
            pi += 1
        nc.vector.tensor_copy(probe[0:BS, pi:pi + 1], I8_sb[:, 0:1])
        pi += 1
        # PE probes: a PE instruction carries at most ONE sync wait, so each
        # weight tensor's DMA sem is absorbed by its own dummy matmul. All
        # probes form ONE accumulation group in the trTp bank so they don't
        # serialize on each other with extra PE-completion waits.
        dummy = ppool.tile([128, (UC + 1) * BS], f32, tag="trTp")
        nc.tensor.matmul(dummy[0:BS, 0:BS], I8_sb[:], I8_sb[:], start=True, stop=True)
        nc.tensor.matmul(
            dummy[0:BS, 0:MP1], Wr_sb[:, 0, 0:BS], Wr_sb[:, 0, :],
            start=True, stop=True,
        )
        nc.tensor.matmul(
            dummy[0:BS, 0:BS], Wt_sb[:, 0, 0:BS], Wt_sb[:, 0, 0:BS],
            start=True, stop=True,
        )
        nc.tensor.matmul(
            dummy[0:M, 0:M], Ww_sb[:, 0, :], Ww_sb[:, 0, :],
            start=True, stop=True,
        )
        nc.tensor.matmul(
            dummy[:, 0:BS], ones8_sb[:], I8_sb[:], start=True, stop=True
        )

        for t in range(tsteps):
            last = t == tsteps - 1
            # x_t into slot 16 (all scan elementwise work lives on VE: the
            # PE/ACT/GPSIMD instruction formats only allow a single sync
            # wait, DVE tolerates several)
            nc.vector.tensor_copy(mem[:, :, :, 16], xs[:, :, :, t])

            # ---- mean over att (sum; the 1/17 is folded into WrS) ----
            m_mean = work.tile([128, UC, BS], f32, tag="mean")
            src = mem[:, :, :, 0:MP1] if t == 0 else mem[:, :, :, 16:49]
            nc.vector.reduce_sum(m_mean[:], src, axis=AX.X)
            if t > 0:
                # mem' = tmp1 + tmp2 (off the critical path: the 17-mean
                # reads slots 16:49, not 0:16; this fills the logits window)
                nc.vector.tensor_add(
                    mem[:, :, :, 0:M], mem[:, :, :, 33:49], mem[:, :, :, 17:33]
                )

            # ---- logits = mean @ WrS  -> psum [8, 17] ----
            lg = ppool.tile([BS, MP1], f32, tag="lg")
            for k in range(UC):
                nc.tensor.matmul(
                    lg[:], m_mean[:, k, :], Wr_sb[:, k, :],
                    start=(k == 0), stop=(k == UC - 1),
                )

            # ---- e = exp(logits) (+ row-sum Z), rz = 1/Z ----
            # psum -> sbuf via VE so the ACT exp never reads PSUM (keeps the
            # next step's lg matmul at a single sync wait)
            lgs = work.tile([BS, MP1], f32, tag="lgs")
            nc.vector.tensor_copy(lgs[:], lg[:])
            e_sb = work.tile([BS, MP1], f32, tag="e")
            zcol = work.tile([BS, 1], f32, tag="z")
            if use_br:
                nc.scalar.activation(e_sb[:], lgs[:], AF.Exp)
                nc.vector.tensor_mul(e_sb[:], e_sb[:], ebr_sb[:])
                nc.vector.reduce_sum(zcol[:], e_sb[:], axis=AX.X)
            else:
                nc.scalar.activation(e_sb[:], lgs[:], AF.Exp, accum_out=zcol[:])
            rz = work.tile([BS, 1], f32, tag="rz")
            nc.vector.reciprocal(rz[:], zcol[:])

            # ---- broadcast e across partitions: Re = e (x) I8; wbc = ones^T Re
            Re = work.tile([BS, BS, MP1], f32, tag="Re")
            nc.vector.tensor_mul(
                Re[:],
                e_sb[:].unsqueeze(1).to_broadcast([BS, BS, MP1]),
                I8_sb[:].unsqueeze(2).to_broadcast([BS, BS, MP1]),
            )
            wbc = ppool.tile([128, BS, MP1], f32, tag="wbc", bufs=2)
            nc.tensor.matmul(wbc[:], ones8_sb[:], Re[:], start=True, stop=True)

            # ---- q = sum_j e_j * att_j (unnormalized read), u-major ----
            qtmp = work.tile([128, UC, BS, MP1], f32, tag="qtmp")
            nc.vector.tensor_mul(
                qtmp[:],
                mem[:, :, :, 0:MP1],
                wbc[:].unsqueeze(1).to_broadcast([128, UC, BS, MP1]),
            )
            qT = work.tile([128, UC, BS], f32, tag="qT")
            nc.vector.reduce_sum(qT[:], qtmp[:], axis=AX.X)

            # ---- trlog = q @ Wt -> psum [8, 512] (batch-major) ----
            trlog = ppool.tile([BS, U], f32, tag="trlog")
            for k in range(UC):
                nc.tensor.matmul(
                    trlog[:], qT[:, k, :], Wt_sb[:, k, :],
                    start=(k == 0), stop=(k == UC - 1),
                )

            # ---- tr = relu(trlog * rz (+ bt)) ----
            # on VE (tensor_scalar handles the per-partition rz and the max;
            # DVE also tolerates several sync waits, unlike ACT/PE)
            trb = work.tile([BS, U], f32, tag="trb")
            if use_bt:
                nc.vector.tensor_scalar_mul(trb[:], trlog[:], rz[:])
                nc.vector.tensor_add(trb[:], trb[:], bt_sb[:])
                nc.vector.tensor_scalar_max(trb[:], trb[:], 0.0)
            else:
                nc.vector.tensor_scalar(
                    trb[:], trlog[:], rz[:], 0.0,
                    op0=mybir.AluOpType.mult, op1=mybir.AluOpType.max,
                )

            # ---- transpose tr to u-major via PE ----
            trTp = ppool.tile([128, UC + 1, BS], f32, tag="trTp")
            # leading nop transpose absorbs the bank/slot PE-completion wait
            # so the first real transpose carries only its RAW wait
            nc.tensor.matmul(
                trTp[:, UC, :], ones8_sb[:], I8_sb[:],
                is_transpose=True, start=True, stop=True,
            )
            for k in range(UC):
                nc.tensor.matmul(
                    trTp[:, k, :], trb[:, k * 128:(k + 1) * 128], I8_sb[:],
                    is_transpose=True, start=True, stop=True,
                )
            trT = work.tile([128, UC, BS], f32, tag="trT")
            nc.vector.tensor_copy(trT[:], trTp[:, 0:UC, :])

            # ---- y_t = x_t + tr ----
            nc.vector.tensor_add(ys[:, :, :, t], xs[:, :, :, t], trT[:])

            if last:
                break

            # ---- ww = sigmoid(tr @ Ww + bw); omw = sigmoid(-(tr @ Ww + bw)) ----
            wwp = ppool.tile([BS, M], f32, tag="wwp")
            for k in range(UC):
                nc.tensor.matmul(
                    wwp[:], trT[:, k, :], Ww_sb[:, k, :],
                    start=(k == 0), stop=(k == UC - 1),
                )
            # sigmoid via exp (stays in the "exp" ACT table set):
            #   a = e^wwlog; ww = a/(1+a); 1-ww = 1/(1+a)
            wws = work.tile([BS, M], f32, tag="wws")
            if use_bw:
                nc.vector.tensor_add(wws[:], wwp[:], bw_sb[:])
            else:
                nc.vector.tensor_copy(wws[:], wwp[:])
            a_sb = work.tile([BS, M], f32, tag="wa")
            nc.scalar.activation(a_sb[:], wws[:], AF.Exp)
            ww2 = work.tile([BS, 2, M], f32, tag="ww2")
            c_sb = work.tile([BS, M], f32, tag="wc")
            nc.vector.tensor_scalar_add(c_sb[:], a_sb[:], 1.0)
            nc.vector.reciprocal(ww2[:, 1, :], c_sb[:])
            nc.vector.tensor_mul(ww2[:, 0, :], a_sb[:], ww2[:, 1, :])

            # ---- broadcast [ww | 1-ww] across partitions ----
            Rw2 = work.tile([BS, 2, BS, M], f32, tag="Rw2")
            nc.vector.tensor_mul(
                Rw2[:],
                ww2[:].unsqueeze(2).to_broadcast([BS, 2, BS, M]),
                I8_sb[:].unsqueeze(1).unsqueeze(3).to_broadcast([BS, 2, BS, M]),
            )
            wb2 = ppool.tile([128, 2, BS, M], f32, tag="wb2", bufs=2)
            nc.tensor.matmul(wb2[:], ones8_sb[:], Rw2[:], start=True, stop=True)

            # ---- mem update halves (merged by next step's 33-slot reduce) ----
            nc.vector.tensor_mul(
                mem[:, :, :, 33:49],
                mem[:, :, :, 0:M],
                wb2[:, 1].unsqueeze(1).to_broadcast([128, UC, BS, M]),
            )
            nc.vector.tensor_mul(
                mem[:, :, :, 17:33],
                trT[:].unsqueeze(3).to_broadcast([128, UC, BS, M]),
                wb2[:, 0].unsqueeze(1).to_broadcast([128, UC, BS, M]),
            )

        # ---- store ys ----
        yT_r = yT[:, :, :].rearrange("(k p) b t -> p k b t", p=128)
        for k in range(UC):
            nc.sync.dma_start(yT_r[:, k, :, :], ys[:, k, :, :])

    nc.compile()
    return nc


def _get_nc(tsteps, use_br, use_bt, use_bw):
    key = (tsteps, use_br, use_bt, use_bw)
    if key not in _NC_CACHE:
        _NC_CACHE[key] = _build(tsteps, use_br, use_bt, use_bw)
    return _NC_CACHE[key]


def _make_in_maps(x, mem0, Wr, br, Wt, bt, Ww, bw, tsteps):
    use_br = bool(np.any(br != 0))
    use_bt = bool(np.any(bt != 0))
    use_bw = bool(np.any(bw != 0))
    f = np.float32
    shared = {
        "WrS": np.ascontiguousarray(Wr, dtype=f) / f(MP1),
        "Wt": np.ascontiguousarray(Wt, dtype=f),
        "Ww": np.ascontiguousarray(Ww, dtype=f),
        "ones8": np.ones((BS, 128), f),
        "I8": np.eye(BS, dtype=f),
    }
    if use_br:
        shared["ebr8"] = np.tile(np.exp(br).astype(f)[None, :], (BS, 1))
    if use_bt:
        shared["bt8"] = np.tile(np.asarray(bt, f)[None, :], (BS, 1))
    if use_bw:
        shared["bw8"] = np.tile(np.asarray(bw, f)[None, :], (BS, 1))
    in_maps = []
    for c in range(NCORES):
        xs = np.asarray(x[c * BS:(c + 1) * BS, :tsteps, :], f)
        ms = np.asarray(mem0[c * BS:(c + 1) * BS], f).reshape(BS, M, U)
        in_maps.append(
            dict(
                shared,
                xT=np.ascontiguousarray(xs.transpose(2, 0, 1)),
                memT=np.ascontiguousarray(ms.transpose(2, 0, 1)),
            )
        )
    return in_maps, use_br, use_bt, use_bw


def kernel(x, mem0, Wr, br, Wt, bt, Ww, bw):
    _ensure_import_paths()
    from concourse.bass_utils import run_bass_kernel_spmd

    x = np.asarray(x, np.float32)
    in_maps, use_br, use_bt, use_bw = _make_in_maps(
        x, mem0, Wr, br, Wt, bt, Ww, bw, T
    )
    nc = _get_nc(T, use_br, use_bt, use_bw)
    res = run_bass_kernel_spmd(nc, in_maps, list(range(NCORES))).results
    out = np.empty((B, T, U), np.float32)
    for c in range(NCORES):
        out[c * BS:(c + 1) * BS] = res[c]["yT"].transpose(1, 2, 0)
    return out


# revision 19
# speedup vs baseline: 2.1830x; 2.1830x over previous
"""DNC-cell scan kernel for Trainium2 (8 NeuronCores, data-parallel over batch).

Reference semantics (per timestep, batch row b):
    att    = concat([mem, x_t[:, None, :]], axis=1)          # [B, M+1, U]
    read_w = softmax(att.mean(1) @ Wr + br)                  # [B, M+1]
    read   = einsum('bm,bmu->bu', read_w, att)               # [B, U]
    tr     = relu(read @ Wt + bt)                            # [B, U]
    ww     = sigmoid(tr @ Ww + bw)[..., None]                # [B, M, 1]
    mem    = (1-ww)*mem + ww*tr[:, None, :]
    y_t    = x_t + tr

Sharding: batch 64 -> 8 cores x 8 rows. All weights replicated; the
time scan stays local per core. Host pre-transposes x/mem0 to U-major
so the kernel needs no on-chip input transposes.

On-chip layout (per core, BS=8 batch rows):
  mem tile  [128(p=u%128), 4(uc), 8(b), 49]  slots: [0:16] mem, [16] x_t,
            [17:33] tmp2 = ww*tr_bc, [33:49] tmp1 = (1-ww)*mem
  The next step's 17-mean reduces slots [16:49] (= x_{t+1} + tmp1 + tmp2),
  which lets mem' = tmp1+tmp2 (u3, on GPSIMD) run off the critical path.

Softmax is computed unnormalized: e = exp(logits); the 1/sum(e) factor is
applied as a per-partition ACT scale on the (batch-major) relu input, since
q @ Wt is linear in q.

Per-partition->free broadcasts use two tricks:
  *  e/ww broadcast across partitions: R = e (x) I8 (one VE op) followed by
     ones[8,128]^T @ R on the PE (sum over the injected identity index
     reproduces e on every partition).
  *  per-b scalars in batch-major layouts are ACT per-partition scales.
"""

import os
import sys
import numpy as np

B, T, U, M = 64, 256, 512, 16
NCORES = 8
BS = B // NCORES          # 8 batch rows per core
UC = U // 128             # 4 partition chunks of the U axis
MP1 = M + 1               # 17

_REPO_CANDIDATES = ("/opt/trn_rl_repo", os.path.expanduser("~/trn_rl_repo"))


def _ensure_import_paths():
    try:
        import concourse  # noqa: F401
        return
    except ImportError:
        pass
    for p in _REPO_CANDIDATES:
        if os.path.isdir(p) and p not in sys.path:
            sys.path.insert(0, p)
    import concourse  # noqa: F401


_NC_CACHE = {}


def _build(tsteps, use_br, use_bt, use_bw):
    """Build the per-core Bass program. Same program runs SPMD on all cores."""
    _ensure_import_paths()
    from contextlib import ExitStack

    import concourse.tile as tile
    from concourse import bacc, mybir

    f32 = mybir.dt.float32
    AF = mybir.ActivationFunctionType
    AX = mybir.AxisListType

    nc = bacc.Bacc()

    xT = nc.declare_dram_parameter("xT", [U, BS, tsteps], f32, isOutput=False)
    memT = nc.declare_dram_parameter("memT", [U, BS, M], f32, isOutput=False)
    WrS = nc.declare_dram_parameter("WrS", [U, MP1], f32, isOutput=False)
    Wt = nc.declare_dram_parameter("Wt", [U, U], f32, isOutput=False)
    Ww = nc.declare_dram_parameter("Ww", [U, M], f32, isOutput=False)
    ones8 = nc.declare_dram_parameter("ones8", [BS, 128], f32, isOutput=False)
    I8 = nc.declare_dram_parameter("I8", [BS, BS], f32, isOutput=False)
    if use_br:
        ebr8 = nc.declare_dram_parameter("ebr8", [BS, MP1], f32, isOutput=False)
    if use_bt:
        bt8 = nc.declare_dram_parameter("bt8", [BS, U], f32, isOutput=False)
    if use_bw:
        bw8 = nc.declare_dram_parameter("bw8", [BS, M], f32, isOutput=False)
    yT = nc.declare_dram_parameter("yT", [U, BS, tsteps], f32, isOutput=True)

    with tile.TileContext(nc) as tc, ExitStack() as ctx:
        const = ctx.enter_context(tc.tile_pool(name="const", bufs=1))
        state = ctx.enter_context(tc.tile_pool(name="state", bufs=1))
        work = ctx.enter_context(tc.tile_pool(name="work", bufs=2))
        ppool = ctx.enter_context(tc.tile_pool(name="ppool", bufs=1, space="PSUM"))

        # ---- constants / weights ----
        Wr_sb = const.tile([128, UC, MP1], f32)
        nc.gpsimd.dma_start(Wr_sb[:], WrS[:, :].rearrange("(k p) j -> p k j", p=128))
        Wt_sb = const.tile([128, UC, U], f32)
        nc.gpsimd.dma_start(Wt_sb[:], Wt[:, :].rearrange("(k p) n -> p k n", p=128))
        Ww_sb = const.tile([128, UC, M], f32)
        nc.gpsimd.dma_start(Ww_sb[:], Ww[:, :].rearrange("(k p) j -> p k j", p=128))
        ones8_sb = const.tile([BS, 128], f32)
        nc.gpsimd.dma_start(ones8_sb[:], ones8[:, :])
        I8_sb = const.tile([BS, BS], f32)
        nc.gpsimd.dma_start(I8_sb[:], I8[:, :])
        if use_br:
            ebr_sb = const.tile([BS, MP1], f32)
            nc.gpsimd.dma_start(ebr_sb[:], ebr8[:, :])
        if use_bt:
            bt_sb = const.tile([BS, U], f32)
            nc.gpsimd.dma_start(bt_sb[:], bt8[:, :])
        if use_bw:
            bw_sb = const.tile([BS, M], f32)
            nc.gpsimd.dma_start(bw_sb[:], bw8[:, :])

        # ---- big state tiles ----
        xs = state.tile([128, UC, BS, tsteps], f32)
        xT_r = xT[:, :, :].rearrange("(k p) b t -> p k b t", p=128)
        for k in range(UC):
            nc.gpsimd.dma_start(xs[:, k, :, :], xT_r[:, k, :, :])
        ys = state.tile([128, UC, BS, tsteps], f32)
        mem = state.tile([128, UC, BS, 49], f32)
        memT_r = memT[:, :, :].rearrange("(k p) b m -> p k b m", p=128)
        for k in range(UC):
            nc.gpsimd.dma_start(mem[:, k, :, 0:M], memT_r[:, k, :, :])

        # Wait-limit workaround: the HW allows only a couple of sync waits
        # per instruction, but a first consumer of several DMA-loaded
        # regions would need one wait per DMA queue. Absorb each DMA's
        # semaphore in its own tiny probe op on every engine that reads
        # the region, so real instructions carry <=2 waits.
        probe = const.tile([128, 16], f32)
        pi = 0
        for k in range(UC):
            nc.vector.tensor_copy(probe[:, pi:pi + 1], xs[:, k, 0, 0:1])
            pi += 1
            nc.vector.tensor_copy(probe[:, pi:pi + 1], mem[:, k, 0, 0:1])
            pi += 1
        nc.vector.tensor_copy(probe[0:BS, pi:pi + 1], I8_sb[:, 0:1])
        pi += 1
        # PE probes: a PE instruction carries at most ONE sync wait, so each
        # weight tensor's DMA sem is absorbed by its own dummy matmul. All
        # probes form ONE accumulation group in the trTp bank so they don't
        # serialize on each other with extra PE-completion waits.
        dummy = ppool.tile([128, (UC + 1) * BS], f32, tag="trTp")
        nc.tensor.matmul(dummy[0:BS, 0:BS], I8_sb[:], I8_sb[:], start=True, stop=True)
        nc.tensor.matmul(
            dummy[0:BS, 0:MP1], Wr_sb[:, 0, 0:BS], Wr_sb[:, 0, :],
            start=True, stop=True,
        )
        nc.tensor.matmul(
            dummy[0:BS, 0:BS], Wt_sb[:, 0, 0:BS], Wt_sb[:, 0, 0:BS],
            start=True, stop=True,
        )
        nc.tensor.matmul(
            dummy[0:M, 0:M], Ww_sb[:, 0, :], Ww_sb[:, 0, :],
            start=True, stop=True,
        )
        nc.tensor.matmul(
            dummy[:, 0:BS], ones8_sb[:], I8_sb[:], start=True, stop=True
        )

        for t in range(tsteps):
            last = t == tsteps - 1
            # x_t into slot 16 (gpsimd, off the VE critical path)
            nc.gpsimd.tensor_copy(mem[:, :, :, 16], xs[:, :, :, t])
            if t > 0:
                # mem' = tmp1 + tmp2 on gpsimd: hidden under the logits/
                # softmax window (the 17-mean below reads slots 16:49)
                nc.gpsimd.tensor_add(
                    mem[:, :, :, 0:M], mem[:, :, :, 33:49], mem[:, :, :, 17:33]
                )

            # ---- mean over att (sum; the 1/17 is folded into WrS) ----
            m_mean = work.tile([128, UC, BS], f32, tag="mean")
            src = mem[:, :, :, 0:MP1] if t == 0 else mem[:, :, :, 16:49]
            nc.vector.reduce_sum(m_mean[:], src, axis=AX.X)

            # ---- logits = mean @ WrS  -> psum [8, 17] ----
            lg = ppool.tile([BS, MP1], f32, tag="lg")
            for k in range(UC):
                nc.tensor.matmul(
                    lg[:], m_mean[:, k, :], Wr_sb[:, k, :],
                    start=(k == 0), stop=(k == UC - 1),
                )

            # ---- e = exp(logits) (+ row-sum Z), rz = 1/Z ----
            e_sb = work.tile([BS, MP1], f32, tag="e")
            zcol = work.tile([BS, 1], f32, tag="z")
            if use_br:
                nc.scalar.activation(e_sb[:], lg[:], AF.Exp)
                nc.vector.tensor_mul(e_sb[:], e_sb[:], ebr_sb[:])
                nc.vector.reduce_sum(zcol[:], e_sb[:], axis=AX.X)
            else:
                nc.scalar.activation(e_sb[:], lg[:], AF.Exp, accum_out=zcol[:])
            rz = work.tile([BS, 1], f32, tag="rz")
            nc.vector.reciprocal(rz[:], zcol[:])

            # ---- broadcast e across partitions: Re = e (x) I8; wbc = ones^T Re
            Re = work.tile([BS, BS, MP1], f32, tag="Re")
            nc.vector.tensor_mul(
                Re[:],
                e_sb[:].unsqueeze(1).to_broadcast([BS, BS, MP1]),
                I8_sb[:].unsqueeze(2).to_broadcast([BS, BS, MP1]),
            )
            wbc = ppool.tile([128, BS, MP1], f32, tag="wbc", bufs=2)
            nc.tensor.matmul(wbc[:], ones8_sb[:], Re[:], start=True, stop=True)

            # ---- q = sum_j e_j * att_j (unnormalized read), u-major ----
            qtmp = work.tile([128, UC, BS, MP1], f32, tag="qtmp")
            nc.vector.tensor_mul(
                qtmp[:],
                mem[:, :, :, 0:MP1],
                wbc[:].unsqueeze(1).to_broadcast([128, UC, BS, MP1]),
            )
            qT = work.tile([128, UC, BS], f32, tag="qT")
            nc.vector.reduce_sum(qT[:], qtmp[:], axis=AX.X)

            # ---- trlog = q @ Wt -> psum [8, 512] (batch-major) ----
            trlog = ppool.tile([BS, U], f32, tag="trlog")
            for k in range(UC):
                nc.tensor.matmul(
                    trlog[:], qT[:, k, :], Wt_sb[:, k, :],
                    start=(k == 0), stop=(k == UC - 1),
                )

            # ---- tr = relu(trlog * rz (+ bt)) ----
            # on VE (tensor_scalar handles the per-partition rz and the max;
            # DVE also tolerates several sync waits, unlike ACT/PE)
            trb = work.tile([BS, U], f32, tag="trb")
            if use_bt:
                nc.vector.tensor_scalar_mul(trb[:], trlog[:], rz[:])
                nc.vector.tensor_add(trb[:], trb[:], bt_sb[:])
                nc.vector.tensor_scalar_max(trb[:], trb[:], 0.0)
            else:
                nc.vector.tensor_scalar(
                    trb[:], trlog[:], rz[:], 0.0,
                    op0=mybir.AluOpType.mult, op1=mybir.AluOpType.max,
                )

            # ---- transpose tr to u-major via PE ----
            trTp = ppool.tile([128, UC + 1, BS], f32, tag="trTp")
            # leading nop transpose absorbs the bank/slot PE-completion wait
            # so the first real transpose carries only its RAW wait
            nc.tensor.matmul(
                trTp[:, UC, :], ones8_sb[:], I8_sb[:],
                is_transpose=True, start=True, stop=True,
            )
            for k in range(UC):
                nc.tensor.matmul(
                    trTp[:, k, :], trb[:, k * 128:(k + 1) * 128], I8_sb[:],
                    is_transpose=True, start=True, stop=True,
                )
            trT = work.tile([128, UC, BS], f32, tag="trT")
            nc.vector.tensor_copy(trT[:], trTp[:, 0:UC, :])

            # ---- y_t = x_t + tr ----
            nc.gpsimd.tensor_add(ys[:, :, :, t], xs[:, :, :, t], trT[:])

            if last:
                break

            # ---- ww = sigmoid(tr @ Ww + bw); omw = sigmoid(-(tr @ Ww + bw)) ----
            wwp = ppool.tile([BS, M], f32, tag="wwp")
            for k in range(UC):
                nc.tensor.matmul(
                    wwp[:], trT[:, k, :], Ww_sb[:, k, :],
                    start=(k == 0), stop=(k == UC - 1),
                )
            # sigmoid via exp (stays in the "exp" ACT table set):
            #   a = e^wwlog; ww = a/(1+a); 1-ww = 1/(1+a)
            # sigmoid via tanh (same ACT table set as exp):
            #   ww = (1+tanh(wwlog/2))/2 ; 1-ww = (1-tanh(wwlog/2))/2
            th = work.tile([BS, M], f32, tag="th")
            if use_bw:
                wws = work.tile([BS, M], f32, tag="wws")
                nc.vector.tensor_add(wws[:], wwp[:], bw_sb[:])
                nc.scalar.activation(th[:], wws[:], AF.Tanh, scale=0.5)
            else:
                nc.scalar.activation(th[:], wwp[:], AF.Tanh, scale=0.5)
            ww2 = work.tile([BS, 2, M], f32, tag="ww2")
            nc.vector.tensor_scalar(
                ww2[:, 0, :], th[:], 0.5, 0.5,
                op0=mybir.AluOpType.mult, op1=mybir.AluOpType.add,
            )
            nc.vector.tensor_scalar(
                ww2[:, 1, :], th[:], -0.5, 0.5,
                op0=mybir.AluOpType.mult, op1=mybir.AluOpType.add,
            )

            # ---- broadcast [ww | 1-ww] across partitions ----
            Rw2 = work.tile([BS, 2, BS, M], f32, tag="Rw2")
            nc.vector.tensor_mul(
                Rw2[:],
                ww2[:].unsqueeze(2).to_broadcast([BS, 2, BS, M]),
                I8_sb[:].unsqueeze(1).unsqueeze(3).to_broadcast([BS, 2, BS, M]),
            )
            wb2 = ppool.tile([128, 2, BS, M], f32, tag="wb2", bufs=2)
            nc.tensor.matmul(wb2[:], ones8_sb[:], Rw2[:], start=True, stop=True)

            # ---- mem update halves (merged by next step's 33-slot reduce) ----
            nc.vector.tensor_mul(
                mem[:, :, :, 33:49],
                mem[:, :, :, 0:M],
                wb2[:, 1].unsqueeze(1).to_broadcast([128, UC, BS, M]),
            )
            nc.vector.tensor_mul(
                mem[:, :, :, 17:33],
                trT[:].unsqueeze(3).to_broadcast([128, UC, BS, M]),
                wb2[:, 0].unsqueeze(1).to_broadcast([128, UC, BS, M]),
            )

        # ---- store ys ----
        yT_r = yT[:, :, :].rearrange("(k p) b t -> p k b t", p=128)
        for k in range(UC):
            nc.sync.dma_start(yT_r[:, k, :, :], ys[:, k, :, :])

    nc.compile()
    return nc


def _get_nc(tsteps, use_br, use_bt, use_bw):
    key = (tsteps, use_br, use_bt, use_bw)
    if key not in _NC_CACHE:
        _NC_CACHE[key] = _build(tsteps, use_br, use_bt, use_bw)
    return _NC_CACHE[key]


def _make_in_maps(x, mem0, Wr, br, Wt, bt, Ww, bw, tsteps):
    use_br = bool(np.any(br != 0))
    use_bt = bool(np.any(bt != 0))
    use_bw = bool(np.any(bw != 0))
    f = np.float32
    shared = {
        "WrS": np.ascontiguousarray(Wr, dtype=f) / f(MP1),
        "Wt": np.ascontiguousarray(Wt, dtype=f),
        "Ww": np.ascontiguousarray(Ww, dtype=f),
        "ones8": np.ones((BS, 128), f),
        "I8": np.eye(BS, dtype=f),
    }
    if use_br:
        shared["ebr8"] = np.tile(np.exp(br).astype(f)[None, :], (BS, 1))
    if use_bt:
        shared["bt8"] = np.tile(np.asarray(bt, f)[None, :], (BS, 1))
    if use_bw:
        shared["bw8"] = np.tile(np.asarray(bw, f)[None, :], (BS, 1))
    in_maps = []
    for c in range(NCORES):
        xs = np.asarray(x[c * BS:(c + 1) * BS, :tsteps, :], f)
        ms = np.asarray(mem0[c * BS:(c + 1) * BS], f).reshape(BS, M, U)
        in_maps.append(
            dict(
                shared,
                xT=np.ascontiguousarray(xs.transpose(2, 0, 1)),
                memT=np.ascontiguousarray(ms.transpose(2, 0, 1)),
            )
        )
    return in_maps, use_br, use_bt, use_bw


def kernel(x, mem0, Wr, br, Wt, bt, Ww, bw):
    _ensure_import_paths()
    from concourse.bass_utils import run_bass_kernel_spmd

    x = np.asarray(x, np.float32)
    in_maps, use_br, use_bt, use_bw = _make_in_maps(
        x, mem0, Wr, br, Wt, bt, Ww, bw, T
    )
    nc = _get_nc(T, use_br, use_bt, use_bw)
    res = run_bass_kernel_spmd(nc, in_maps, list(range(NCORES))).results
    out = np.empty((B, T, U), np.float32)
    for c in range(NCORES):
        out[c * BS:(c + 1) * BS] = res[c]["yT"].transpose(1, 2, 0)
    return out
